# revision 1
# baseline (speedup 1.0000x reference)
"""Trainium2 Bass kernel for nn_CrossAttention (per-head-LN cross attention).

Sharding: 16 heads / 8 cores -> 2 heads per core, both batch elements on every
core (attention is embarrassingly parallel over (B, H)). Each core computes its
128 output channels [128p, 128p+128) of the final [S, B, 1024] output.

Device algorithm (per core, all matmuls bf16 with f32 PSUM accumulation):
  - Projections computed transposed: Y^T[o, t] (o = core's 128 channels,
    t = b*S + s), via stationary W^T chunks against streamed X^T tiles.
  - Per-head mean subtraction is folded into the weights on the host
    (W' = W - per-head column mean), so Y^T is already centered: no mu
    matmul or subtract on device.  Variance via matmul-broadcast:
    var_bc = blockones.T @ yt^2 (bf16), rstd via ACT Sqrt + DVE reciprocal.
    The 1/sqrt(head) score scale is folded into Q's Sqrt scale.
  - V transposed back to natural [k, d] per (b,h) via PE transposes, with a
    ones column appended (row 64 of the PV output then holds the softmax
    denominator).
  - scores^T[k, q] = K^T.T @ Q^T per (b,h); softmax without max subtraction
    (scores are O(1) after LN; exp cannot overflow); exp on ACT directly from
    PSUM; PV: out^T[d|den, q] accumulated over k chunks; PE-transpose (bf16)
    back to natural [q, d] and multiply by 1/den.
  - Phases are fused: batch-0 projections first, then batch-0 attention with
    batch-1 projections inserted as two clusters between attention units
    (keeps ACT exp/sqrt table switches to ~4 and hides the batch-1 HBM loads
    under attention compute).  Output stores are issued per (b, qc) unit.
"""

import os
import numpy as np
import ml_dtypes

import concourse.bacc as bacc
import concourse.mybir as mybir
import concourse.tile as tile
from concourse.bass_utils import run_bass_kernel_spmd

F32 = mybir.dt.float32
BF16 = mybir.dt.bfloat16
AF = mybir.ActivationFunctionType
ALU = mybir.AluOpType

S = 2048
B = 2
DIM = 1024
NHEAD = 16
HEAD = 64
EPS = 1e-5
NCORES = 8
OC = DIM // NCORES          # 128 output channels per core
HPC = OC // HEAD            # 2 heads per core
T = S * B                   # 4096 tokens (t = b*S + s)
TCH = 512                   # token chunk (matmul moving free dim)
NT = T // TCH               # 8 token chunks (0-3 = b0, 4-7 = b1)
NCC = DIM // 128            # 8 contraction chunks
QCH = 512
NQ = S // QCH               # 4 q chunks per (b, h)
NKT = S // 128              # 16 k tiles per (b, h)
N_FIN = HPC * (QCH // 128)  # 8 transpose+normalize items per unit

LAST_RESULT = None


def _emit(tc, aps, flags):
    from contextlib import ExitStack

    nc = tc.nc
    names = ("q", "k", "v")

    stack = ExitStack()
    consts = stack.enter_context(tc.tile_pool(name="consts", bufs=1))
    persist = stack.enter_context(tc.tile_pool(name="persist", bufs=1))
    xload = stack.enter_context(tc.tile_pool(name="xload", bufs=3))
    p1tmp = stack.enter_context(tc.tile_pool(name="p1tmp", bufs=2))
    lnp = stack.enter_context(tc.tile_pool(name="lnpool", bufs=1))
    attn_pool = stack.enter_context(tc.tile_pool(name="attn", bufs=2))
    p3tmp = stack.enter_context(tc.tile_pool(name="p3tmp", bufs=1))
    ps_y_pool = stack.enter_context(tc.tile_pool(name="ps_y", bufs=1, space="PSUM"))
    ps_wave_pool = stack.enter_context(
        tc.tile_pool(name="ps_wave", bufs=2, space="PSUM")
    )
    ps_o_pool = stack.enter_context(tc.tile_pool(name="ps_o", bufs=2, space="PSUM"))
    ps_misc_pool = stack.enter_context(
        tc.tile_pool(name="ps_misc", bufs=1, space="PSUM")
    )

    # Pin the ACT function table to the set containing BOTH Exp and Ln so
    # the auto-inserter never ping-pongs 1.3us table reloads between the
    # attention exp stream and the LN rstd ln/exp ops.
    from concourse.hw_specs import get_activation_tables

    tabs = get_activation_tables(nc.m.arch)
    combined_idx = None
    for i, (_name, fns) in enumerate(tabs.items()):
        if AF.Exp in fns and AF.Ln in fns:
            combined_idx = i
            break
    assert combined_idx is not None, "no ACT table with both Exp and Ln"
    nc.scalar.add_instruction(
        mybir.InstLoadActFuncSet(
            name=nc.get_next_instruction_name(),
            act_func_set_id=combined_idx,
            ins=[], outs=[],
        )
    )

    # ---------------- constants + critical-path input loads ----------------
    # DMA issue order tracks first use: the k-pair-0 tokens and k weights
    # gate the first projection matmul, so they go first; everything else
    # interleaves behind them.
    xt_tiles = {}

    def load_xt(n, pair):
        xt = xload.tile([128, NCC, 2 * TCH], BF16, tag="xt", name="xt")
        psl = slice(pair * 2 * TCH, (pair + 1) * 2 * TCH)
        nc.sync.dma_start(
            out=xt,
            in_=aps[f"xt_{n}"][:, psl].rearrange("(a p) t -> p a t", p=128),
        )
        xt_tiles[(n, pair)] = xt

    wt_sb = {}

    def load_wt(n):
        t = consts.tile([128, NCC, OC], BF16, tag=f"wt_{n}", name=f"wt_{n}")
        nc.sync.dma_start(out=t, in_=aps[f"wt_{n}"])
        wt_sb[n] = t

    load_xt("k", 0)
    load_wt("k")
    load_xt("k", 1)
    load_wt("v")
    bones16 = consts.tile([128, OC], BF16, tag="bones16", name="bones16")
    nc.sync.dma_start(out=bones16, in_=aps["blockones_bf16"])
    load_xt("v", 0)
    load_xt("v", 1)
    load_wt("q")
    id16 = consts.tile([128, 128], BF16, tag="id16", name="id16")
    nc.sync.dma_start(out=id16, in_=aps["identity_bf16"])
    load_xt("q", 0)
    eps_q = consts.tile([128, 1], F32, tag="eps_q", name="eps_q")
    nc.vector.memset(eps_q, float(HEAD * EPS))
    eps_kv = consts.tile([128, 1], F32, tag="eps_kv", name="eps_kv")
    nc.vector.memset(eps_kv, float(EPS))
    extra = {}
    for n in names:
        if flags[f"bias_{n}"]:
            t = consts.tile([128, 1], F32, tag=f"bcol_{n}", name=f"bcol_{n}")
            nc.sync.dma_start(out=t, in_=aps[f"bcol_{n}"])
            extra[f"bcol_{n}"] = t
        if flags[f"gb_{n}"]:
            tg = consts.tile([128, 1], F32, tag=f"gcol_{n}", name=f"gcol_{n}")
            nc.sync.dma_start(out=tg, in_=aps[f"gcol_{n}"])
            tb = consts.tile([128, 1], F32, tag=f"betacol_{n}", name=f"betacol_{n}")
            nc.sync.dma_start(out=tb, in_=aps[f"betacol_{n}"])
            extra[f"gcol_{n}"] = tg
            extra[f"betacol_{n}"] = tb

    # ---------------- persistent tiles ----------------
    ln_sb = {
        n: lnp.tile([128, T], BF16, tag=f"ln_{n}", name=f"ln_{n}") for n in names
    }
    vnat = {}
    for hl in range(HPC):
        v = persist.tile(
            [128, B, NKT, HEAD + 1], BF16, tag=f"vnat{hl}", name=f"vnat{hl}"
        )
        nc.vector.memset(v[:, :, :, HEAD:HEAD + 1], 1.0)
        vnat[hl] = v
    ostage = persist.tile([128, B, NKT, OC], F32, tag="ostage", name="ostage")

    # ---------------- projection + LN machinery ----------------
    state = {}
    stats_q = []
    norm_q = []
    v_normed = []   # chunks of v whose norm has been emitted
    vt_done = set()  # v chunks whose transposes have been emitted

    def emit_proj(n, c, queue=True):
        pair, half = c // 2, c % 2
        if (n, pair) not in xt_tiles:
            load_xt(n, pair)
        xt = xt_tiles[(n, pair)]
        ps_y = ps_y_pool.tile([128, TCH], F32, tag="ps_y", name="ps_y")
        for cc in range(NCC):
            nc.tensor.matmul(
                ps_y, lhsT=wt_sb[n][:, cc, :],
                rhs=xt[:, cc, half * TCH:(half + 1) * TCH],
                start=(cc == 0), stop=(cc == NCC - 1),
            )
        # yt stays alive from the proj fill step until its (lagged) norm
        # fill step — a too-small ring creates a PE->DVE->ACT->PE circular
        # wait (deadlock), so keep generous slack over the ~4 alive.
        yt = p1tmp.tile([128, TCH], BF16, tag="yt", name="yt", bufs=8)
        if flags[f"bias_{n}"]:
            nc.vector.tensor_scalar(
                out=yt, in0=ps_y, scalar1=extra[f"bcol_{n}"],
                scalar2=None, op0=ALU.add,
            )
        else:
            nc.vector.tensor_copy(out=yt, in_=ps_y)
        state[(n, c)] = yt
        if queue:
            stats_q.append((n, c))

    def emit_stats(n, c):
        yt = state[(n, c)]
        sq = p1tmp.tile([128, TCH], BF16, tag="sq", name="sq")
        nc.vector.tensor_mul(sq, yt, yt)
        ps_var = ps_misc_pool.tile([128, TCH], F32, tag="misc", name="ps_var")
        nc.tensor.matmul(ps_var, lhsT=bones16, rhs=sq, start=True, stop=True)
        state[(n, c, "v")] = ps_var

    def emit_norm(n, c):
        # rstd = exp(-0.5*ln(var + eps)): both ln and exp live in the same
        # ACT table (natural_log_exp), so LN work interleaves freely with the
        # attention exp stream without 1.5us table reloads. The q head also
        # folds the 1/sqrt(HEAD) score scale: ln(HEAD*var + HEAD*eps) adds
        # ln(HEAD), so exp(-0.5*...) yields rstd/sqrt(HEAD).
        ps_var = state.pop((n, c, "v"))
        yt = state.pop((n, c))
        lnv = p1tmp.tile([128, TCH], F32, tag="lnv", name="lnv")
        if n == "q":
            nc.scalar.activation(lnv, ps_var, AF.Ln, bias=eps_q, scale=float(HEAD))
        else:
            nc.scalar.activation(lnv, ps_var, AF.Ln, bias=eps_kv, scale=1.0)
        s_t = p1tmp.tile([128, TCH], F32, tag="s_t", name="s_t")
        nc.scalar.activation(s_t, lnv, AF.Exp, scale=-0.5)
        tsl = slice(c * TCH, (c + 1) * TCH)
        if flags[f"gb_{n}"]:
            lnf = p1tmp.tile([128, TCH], F32, tag="lnf", name="lnf")
            nc.vector.tensor_mul(lnf, yt, s_t)
            nc.vector.tensor_scalar(
                out=ln_sb[n][:, tsl], in0=lnf,
                scalar1=extra[f"gcol_{n}"], scalar2=extra[f"betacol_{n}"],
                op0=ALU.mult, op1=ALU.add,
            )
        else:
            nc.vector.tensor_mul(ln_sb[n][:, tsl], yt, s_t)
        if n == "v":
            v_normed.append(c)

    def advance():
        if len(stats_q) > 2:
            u = stats_q.pop(0)
            emit_stats(*u)
            norm_q.append(u)
        if len(norm_q) > 1:
            emit_norm(*norm_q.pop(0))

    def flush():
        while stats_q:
            u = stats_q.pop(0)
            emit_stats(*u)
            norm_q.append(u)
            if len(norm_q) > 1:
                emit_norm(*norm_q.pop(0))
        while norm_q:
            emit_norm(*norm_q.pop(0))

    def emit_vtrans(b, kt, hl):
        # one V tile back to natural layout: ln_v[64d, 128k] -> vnat[128k, 64d]
        dsl = slice(HEAD * hl, HEAD * (hl + 1))
        t0 = b * S
        ps_tr = ps_misc_pool.tile([128, HEAD], BF16, tag="misc", name="ps_tr")
        nc.tensor.transpose(
            ps_tr,
            ln_sb["v"][dsl, t0 + kt * 128: t0 + (kt + 1) * 128],
            id16[dsl, dsl],
        )
        nc.vector.tensor_copy(out=vnat[hl][:, b, kt, 0:HEAD], in_=ps_tr)

    def sweep_vtrans_b0():
        for c in list(v_normed):
            if c < 4 and c not in vt_done:
                vt_done.add(c)
                for ktl in range(4):
                    kt = (c % 4) * 4 + ktl
                    for hl in range(HPC):
                        emit_vtrans(0, kt, hl)

    # ---------------- attention machinery ----------------
    pend = {"pv": None, "fin": None}

    def emit_pv(pu, kt):
        for hl in range(HPC):
            nc.tensor.matmul(
                pu["ps_o"][hl],
                lhsT=vnat[hl][:, pu["b"], kt, :],
                rhs=pu["at_q"][:, kt, hl, :],
                start=(kt == 0), stop=(kt == NKT - 1),
            )

    def emit_fin_a_hl(pu, hl):
        # DVE-only stage: drain ps_o[hl] into oT bf16 (rows 0..63 = out^T
        # values, row 64 = 1/den; row 65 pads the transpose to an even
        # bf16 element count and is never read). Frees ps_o quickly.
        oT = p3tmp.tile([HEAD + 2, QCH], BF16, tag="oT", name="oT", bufs=4)
        nc.vector.tensor_copy(out=oT[:HEAD, :], in_=pu["ps_o"][hl][:HEAD, :])
        # reciprocal_approx_fast requires base_partition 0 on both
        # operands (HW uop quirk) — stage the den row through base 0.
        den = p3tmp.tile([1, QCH], F32, tag="den", name="den", bufs=2)
        nc.vector.tensor_copy(out=den, in_=pu["ps_o"][hl][HEAD:HEAD + 1, :])
        inv = p3tmp.tile([1, QCH], F32, tag="inv", name="inv", bufs=2)
        nc.vector.reciprocal_approx_fast(inv, den)
        nc.vector.tensor_copy(out=oT[HEAD:HEAD + 1, :], in_=inv)
        pu.setdefault("oT", {})[hl] = oT

    def emit_fin_a(pu):
        for hl in range(HPC):
            emit_fin_a_hl(pu, hl)

    def emit_fin_b_item(pu, item, pool=None):
        # one transpose + normalize: item indexes (hl, sub)
        hl, sub = item // (QCH // 128), item % (QCH // 128)
        b, qc = pu["b"], pu["qc"]
        if pool is None:
            pool = ps_misc_pool
            tag = "misc"
        else:
            tag = "wave"
        ps_tro = pool.tile([128, HEAD + 2], BF16, tag=tag, name="ps_tro")
        nc.tensor.transpose(
            ps_tro, pu["oT"][hl][:, sub * 128:(sub + 1) * 128],
            id16[:HEAD + 2, :HEAD + 2],
        )
        # tensor_scalar needs an f32 scalar; stage the bf16 inv column
        inv_col = p3tmp.tile([128, 1], F32, tag="invc", name="inv_col", bufs=2)
        nc.vector.tensor_copy(out=inv_col, in_=ps_tro[:, HEAD:HEAD + 1])
        nc.vector.tensor_scalar(
            out=ostage[:, b, qc * (QCH // 128) + sub, HEAD * hl:HEAD * (hl + 1)],
            in0=ps_tro[:, 0:HEAD],
            scalar1=inv_col,
            scalar2=None, op0=ALU.mult,
        )

    def emit_store(pu):
        b, qc = pu["b"], pu["qc"]
        nsl = slice(qc * (QCH // 128), (qc + 1) * (QCH // 128))
        dst = aps["out"][:, b, :].rearrange("(n p) c -> p n c", p=128)[:, nsl, :]
        nc.sync.dma_start(out=dst, in_=ostage[:, b, nsl, :])

    def attn_unit(b, qc, vt_items=(), fills=()):
        t0 = b * S
        at_q = attn_pool.tile(
            [128, NKT, HPC, QCH], BF16, tag="at", name="at_q"
        )
        if pend["pv"] is not None:
            pend["pv"]["ps_o"] = [
                ps_o_pool.tile(
                    [HEAD + 1, QCH], F32, tag="ps_o", name="ps_o"
                )
                for _ in range(HPC)
            ]
        vt_items = list(vt_items)
        fills = list(fills)
        for kt in range(NKT):
            ps_wave = ps_wave_pool.tile(
                [128, HPC, QCH], F32, tag="wave", name="ps_wave"
            )
            for hl in range(HPC):
                dsl = slice(HEAD * hl, HEAD * (hl + 1))
                nc.tensor.matmul(
                    ps_wave[:, hl, :],
                    lhsT=ln_sb["k"][dsl, t0 + kt * 128: t0 + (kt + 1) * 128],
                    rhs=ln_sb["q"][dsl, t0 + qc * QCH: t0 + (qc + 1) * QCH],
                    start=True, stop=True,
                )
            nc.scalar.activation(at_q[:, kt], ps_wave, AF.Exp)
            if pend["pv"] is not None:
                emit_pv(pend["pv"], kt)
            if pend["fin"] is not None and kt < N_FIN:
                emit_fin_b_item(pend["fin"], kt)
            for _ in range(2):
                if vt_items:
                    emit_vtrans(*vt_items.pop(0))
            if fills:
                fills.pop(0)()
        assert not vt_items
        assert not fills
        if pend["fin"] is not None:
            emit_store(pend["fin"])
        if pend["pv"] is not None:
            emit_fin_a(pend["pv"])
        pend["fin"] = pend["pv"]
        pend["pv"] = {"b": b, "qc": qc, "at_q": at_q}

    # ---------------- fused schedule ----------------
    # phase A: batch-0 projections (k, v first so b0 attention can start
    # as soon as q0/q1 land), with V transposes swept in as norms complete.
    for u in [("k", 0), ("k", 1), ("k", 2), ("k", 3),
              ("v", 0), ("v", 1), ("v", 2), ("v", 3),
              ("q", 0), ("q", 1)]:
        emit_proj(*u)
        advance()
        sweep_vtrans_b0()
    flush()
    sweep_vtrans_b0()

    # Remaining projections run during b0 attention: the PE work (proj/var
    # matmuls), the DVE chains, and the ln/exp-based norms are spread one
    # step per wave inside the attention units.
    def fills_for(units):
        # A(i) = proj matmuls + yt copy; B(i) = square + var matmul + norm.
        # B lags A by two steps so the DVE/ACT chain is never on the PE's
        # critical path.
        out = []
        for i, u in enumerate(units):
            out.append(("A", u))
            if i >= 2:
                out.append(("B", units[i - 2]))
        out.append(("B", units[-2]))
        out.append(("B", units[-1]))

        def mk(step):
            kind, u = step
            if kind == "A":
                return lambda: emit_proj(*u, queue=False)
            return lambda: (emit_stats(*u), emit_norm(*u))

        return [mk(s) for s in out]

    c1 = [("q", 2), ("q", 3), ("k", 4), ("k", 5), ("k", 6), ("k", 7),
          ("v", 4), ("v", 5)]
    c2 = [("v", 6), ("v", 7), ("q", 4), ("q", 5), ("q", 6), ("q", 7)]
    f1 = fills_for(c1)
    f2 = fills_for(c2)

    attn_unit(0, 0, fills=f1[:len(f1) // 2])
    attn_unit(0, 1, fills=f1[len(f1) // 2:])
    attn_unit(0, 2, fills=f2[:len(f2) // 2])
    attn_unit(0, 3, fills=f2[len(f2) // 2:])
    # batch-1 V transposes ride inside the first b1 attention unit's waves
    vt_b1 = [
        (1, (c % 4) * 4 + ktl, hl)
        for c in (4, 5, 6, 7) for ktl in range(4) for hl in range(HPC)
    ]
    attn_unit(1, 0, vt_items=vt_b1)
    attn_unit(1, 1)
    attn_unit(1, 2)
    attn_unit(1, 3)

    # drain: PV + finish of the last two units. The last unit's transposes
    # use the (now idle) ps_wave slots so consecutive items double-buffer,
    # and each head's transposes start as soon as that head's oT is drained.
    pend["pv"]["ps_o"] = [
        ps_o_pool.tile([HEAD + 1, QCH], F32, tag="ps_o", name="ps_o")
        for _ in range(HPC)
    ]
    for kt in range(NKT):
        emit_pv(pend["pv"], kt)
        if pend["fin"] is not None and kt < N_FIN:
            emit_fin_b_item(pend["fin"], kt)
    if pend["fin"] is not None:
        emit_store(pend["fin"])
    emit_fin_a_hl(pend["pv"], 0)
    for item in range(N_FIN // 2):
        if item == 0:
            emit_fin_a_hl(pend["pv"], 1)
        emit_fin_b_item(pend["pv"], item, pool=ps_wave_pool)
    for item in range(N_FIN // 2, N_FIN):
        emit_fin_b_item(pend["pv"], item, pool=ps_wave_pool)
    emit_store(pend["pv"])

    stack.close()


def _build(flags_key, flags, input_specs):
    nc = bacc.Bacc("TRN2", target_bir_lowering=False, debug=False)
    aps = {}
    for name, shape, dt in input_specs:
        aps[name] = nc.dram_tensor(name, list(shape), dt, kind="ExternalInput").ap()
    aps["out"] = nc.dram_tensor("out", [S, B, OC], F32, kind="ExternalOutput").ap()
    with tile.TileContext(nc) as tc:
        _emit(tc, aps, flags)
    nc.compile()
    return nc


_CACHE = {}


def kernel(**inputs):
    global LAST_RESULT
    bf16 = ml_dtypes.bfloat16
    f32 = np.float32

    Q, K, V = (np.asarray(inputs[n], f32) for n in ("Q", "K", "V"))
    W = {n: np.asarray(inputs["W" + n.upper()], f32) for n in ("q", "k", "v")}
    bias = {n: np.asarray(inputs["b" + n.upper()], f32) for n in ("q", "k", "v")}
    g = {n: np.asarray(inputs["g" + n.upper()], f32) for n in ("q", "k", "v")}
    beta = {n: np.asarray(inputs["beta" + n.upper()], f32) for n in ("q", "k", "v")}

    # X^T [c, t] with t = b*S + s
    xt = {
        "q": np.ascontiguousarray(Q.transpose(2, 1, 0).reshape(DIM, T)).astype(bf16),
        "k": np.ascontiguousarray(K.transpose(2, 1, 0).reshape(DIM, T)).astype(bf16),
        "v": np.ascontiguousarray(V.transpose(2, 1, 0).reshape(DIM, T)).astype(bf16),
    }
    blockones = np.kron(np.eye(2, dtype=f32), np.ones((HEAD, HEAD), f32)) / HEAD
    ident = np.eye(128, dtype=f32)

    flags = {}
    for n in ("q", "k", "v"):
        flags[f"bias_{n}"] = bool(np.any(bias[n] != 0.0))
        flags[f"gb_{n}"] = bool(np.any(g[n] != 1.0) or np.any(beta[n] != 0.0))
    flags_key = tuple(sorted(flags.items()))

    # per-core input maps
    in_maps = []
    shared = {
        "xt_q": xt["q"], "xt_k": xt["k"], "xt_v": xt["v"],
        "blockones_bf16": blockones.astype(bf16),
        "identity_bf16": ident.astype(bf16),
    }
    for p in range(NCORES):
        sl = slice(OC * p, OC * (p + 1))
        m = dict(shared)
        for n in ("q", "k", "v"):
            # fold the per-head mean subtraction into the weights: the 64
            # output rows of each head get their column-mean subtracted, so
            # the projection output is already centered.
            Wc = W[n][sl].astype(np.float64)
            Wc = Wc - Wc.reshape(HPC, HEAD, DIM).mean(axis=1, keepdims=True).repeat(
                HEAD, axis=1
            ).reshape(OC, DIM)
            # stationary W^T prepacked as [128, NCC, OC] (partition = c within
            # chunk) so the weight load is one contiguous 2KB-per-line DMA.
            wt = np.ascontiguousarray(
                Wc.T.reshape(NCC, 128, OC).transpose(1, 0, 2)
            ).astype(bf16)
            m[f"wt_{n}"] = wt
            if flags[f"bias_{n}"]:
                bc = bias[n][sl].astype(np.float64)
                bc = bc - bc.reshape(HPC, HEAD).mean(axis=1, keepdims=True).repeat(
                    HEAD, axis=1
                ).reshape(OC)
                m[f"bcol_{n}"] = bc.astype(f32).reshape(128, 1)
            if flags[f"gb_{n}"]:
                m[f"gcol_{n}"] = np.tile(g[n], HPC).astype(f32).reshape(128, 1)
                bcol = np.tile(beta[n], HPC).astype(f32)
                if n == "q":
                    bcol = (bcol / np.sqrt(HEAD)).astype(f32)
                m[f"betacol_{n}"] = bcol.reshape(128, 1)
        in_maps.append(m)

    if flags_key not in _CACHE:
        input_specs = []
        for name, arr in in_maps[0].items():
            dt = BF16 if arr.dtype == bf16 else F32
            input_specs.append((name, arr.shape, dt))
        _CACHE[flags_key] = _build(flags_key, flags, input_specs)
    nc = _CACHE[flags_key]

    trace = bool(os.environ.get("KERNEL_TRACE"))
    tmpdir = os.environ.get("KERNEL_TRACE_DIR") or None
    res = run_bass_kernel_spmd(
        nc, in_maps, core_ids=list(range(NCORES)), trace=trace, tmpdir=tmpdir
    )
    LAST_RESULT = res
    out = np.concatenate(
        [np.asarray(res.results[p]["out"], f32) for p in range(NCORES)], axis=2
    )
    return out



# revision 8
# speedup vs baseline: 1.0871x; 1.0871x over previous
"""Trainium2 Bass kernel for nn_CrossAttention (per-head-LN cross attention).

Sharding: 16 heads / 8 cores -> 2 heads per core, both batch elements on every
core (attention is embarrassingly parallel over (B, H)). Each core computes its
128 output channels [128p, 128p+128) of the final [S, B, 1024] output.

Device algorithm (per core, all matmuls bf16 with f32 PSUM accumulation):
  - Projections computed transposed: Y^T[o, t] (o = core's 128 channels,
    t = b*S + s), via stationary W^T chunks against streamed X^T tiles.
  - Per-head mean subtraction is folded into the weights on the host
    (W' = W - per-head column mean), so Y^T is already centered: no mu
    matmul or subtract on device.  Variance via matmul-broadcast:
    var_bc = blockones.T @ yt^2 (bf16), rstd = exp(-0.5 ln(var+eps)) on ACT.
    The 1/sqrt(head) score scale is folded into Q's rstd.
  - V transposed back to natural [k, d] per (b,h) via PE transposes, with a
    ones column appended (row 64 of the PV output then holds the softmax
    denominator).
  - scores^T[k, q] = K^T.T @ Q^T per (b,h); softmax without max subtraction
    (scores are O(1) after LN; exp cannot overflow); exp on ACT directly from
    PSUM; PV: out^T[d|den, q] accumulated over k chunks; PE-transpose (bf16)
    back to natural [q, d] and multiply by 1/den.
  - Schedule: attention unit (b0,qc0) starts as soon as K chunk0/1 + Q chunk0
    are normed (~8us in).  ALL remaining 21 projection chunks ride as fills
    inside the 8 attention units (1 fill step per wave), with per-chunk DMA
    prefetch 4 A-steps ahead.  V transposes are emitted as per-kt pairs
    (2 PE transposes + 1 DVE copy into a [128, 2hl, ...] vnat tile) gated on
    an earliest-wave so they always follow their chunk's norm in DVE order.
    Output finish transposes are batched in pairs per wave.
"""

import os
import numpy as np
import ml_dtypes

import concourse.bacc as bacc
import concourse.mybir as mybir
import concourse.tile as tile
from concourse.bass_utils import run_bass_kernel_spmd

F32 = mybir.dt.float32
BF16 = mybir.dt.bfloat16
AF = mybir.ActivationFunctionType
ALU = mybir.AluOpType

S = 2048
B = 2
DIM = 1024
NHEAD = 16
HEAD = 64
EPS = 1e-5
NCORES = 8
OC = DIM // NCORES          # 128 output channels per core
HPC = OC // HEAD            # 2 heads per core
T = S * B                   # 4096 tokens (t = b*S + s)
TCH = 512                   # token chunk (matmul moving free dim)
NT = T // TCH               # 8 token chunks per tensor (0-3 = b0, 4-7 = b1)
NCC = DIM // 128            # 8 contraction chunks
QCH = 512
NQ = S // QCH               # 4 q chunks per (b, h)
NKT = S // 128              # 16 k tiles per (b, h)

LAST_RESULT = None

# global A-step order (for DMA prefetch): phase A then per-unit fills
AORDER = [
    ("k", 0), ("k", 1), ("q", 0),                         # phase A
    ("q", 1), ("k", 2), ("v", 0), ("k", 3), ("v", 1),      # u0
    ("v", 2), ("v", 3), ("q", 2),                          # u1
    ("q", 3), ("k", 4), ("k", 5),                          # u2
    ("q", 4), ("k", 6), ("k", 7), ("v", 4), ("v", 5),      # u3
    ("q", 5), ("v", 6), ("v", 7),                          # u4
    ("q", 6),                                              # u5
    ("q", 7),                                              # u6
]
PREFETCH = 4


def _emit(tc, aps, flags):
    from contextlib import ExitStack

    nc = tc.nc
    names = ("q", "k", "v")

    stack = ExitStack()
    consts = stack.enter_context(tc.tile_pool(name="consts", bufs=1))
    persist = stack.enter_context(tc.tile_pool(name="persist", bufs=1))
    xload = stack.enter_context(tc.tile_pool(name="xload", bufs=5))
    p1tmp = stack.enter_context(tc.tile_pool(name="p1tmp", bufs=2))
    lnp = stack.enter_context(tc.tile_pool(name="lnpool", bufs=1))
    attn_pool = stack.enter_context(tc.tile_pool(name="attn", bufs=2))
    p3tmp = stack.enter_context(tc.tile_pool(name="p3tmp", bufs=1))
    ps_y_pool = stack.enter_context(tc.tile_pool(name="ps_y", bufs=1, space="PSUM"))
    ps_wave_pool = stack.enter_context(
        tc.tile_pool(name="ps_wave", bufs=2, space="PSUM")
    )
    ps_o_pool = stack.enter_context(tc.tile_pool(name="ps_o", bufs=2, space="PSUM"))
    ps_misc_pool = stack.enter_context(
        tc.tile_pool(name="ps_misc", bufs=1, space="PSUM")
    )

    # Pin the ACT function table to the set containing BOTH Exp and Ln so
    # the auto-inserter never ping-pongs 1.3us table reloads between the
    # attention exp stream and the LN rstd ln/exp ops.
    from concourse.hw_specs import get_activation_tables

    tabs = get_activation_tables(nc.m.arch)
    combined_idx = None
    for i, (_name, fns) in enumerate(tabs.items()):
        if AF.Exp in fns and AF.Ln in fns:
            combined_idx = i
            break
    assert combined_idx is not None, "no ACT table with both Exp and Ln"
    nc.scalar.add_instruction(
        mybir.InstLoadActFuncSet(
            name=nc.get_next_instruction_name(),
            act_func_set_id=combined_idx,
            ins=[], outs=[],
        )
    )

    # ---------------- constants + critical-path input loads ----------------
    xt_tiles = {}

    def load_xt(n, c):
        if (n, c) in xt_tiles:
            return
        xt = xload.tile([128, NCC, TCH], BF16, tag="xt", name="xt")
        tsl = slice(c * TCH, (c + 1) * TCH)
        nc.sync.dma_start(
            out=xt,
            in_=aps[f"xt_{n}"][:, tsl].rearrange("(a p) t -> p a t", p=128),
        )
        xt_tiles[(n, c)] = xt

    wt_sb = {}

    def load_wt(n):
        t = consts.tile([128, NCC, OC], BF16, tag=f"wt_{n}", name=f"wt_{n}")
        nc.sync.dma_start(out=t, in_=aps[f"wt_{n}"])
        wt_sb[n] = t

    # DMA issue order tracks first use: k chunk0 + k weights gate the first
    # projection matmul, then q0 for the first attention wave.
    load_wt("k")
    load_xt("k", 0)
    bones16 = consts.tile([128, OC], BF16, tag="bones16", name="bones16")
    nc.sync.dma_start(out=bones16, in_=aps["blockones_bf16"])
    load_wt("q")
    load_xt("k", 1)
    load_xt("q", 0)
    id16 = consts.tile([128, 128], BF16, tag="id16", name="id16")
    nc.sync.dma_start(out=id16, in_=aps["identity_bf16"])
    load_wt("v")
    load_xt("q", 1)
    eps_q = consts.tile([128, 1], F32, tag="eps_q", name="eps_q")
    nc.vector.memset(eps_q, float(HEAD * EPS))
    eps_kv = consts.tile([128, 1], F32, tag="eps_kv", name="eps_kv")
    nc.vector.memset(eps_kv, float(EPS))
    extra = {}
    for n in names:
        if flags[f"bias_{n}"]:
            t = consts.tile([128, 1], F32, tag=f"bcol_{n}", name=f"bcol_{n}")
            nc.sync.dma_start(out=t, in_=aps[f"bcol_{n}"])
            extra[f"bcol_{n}"] = t
        if flags[f"gb_{n}"]:
            tg = consts.tile([128, 1], F32, tag=f"gcol_{n}", name=f"gcol_{n}")
            nc.sync.dma_start(out=tg, in_=aps[f"gcol_{n}"])
            tb = consts.tile([128, 1], F32, tag=f"betacol_{n}", name=f"betacol_{n}")
            nc.sync.dma_start(out=tb, in_=aps[f"betacol_{n}"])
            extra[f"gcol_{n}"] = tg
            extra[f"betacol_{n}"] = tb

    # ---------------- persistent tiles ----------------
    ln_sb = {
        n: lnp.tile([128, T], BF16, tag=f"ln_{n}", name=f"ln_{n}") for n in names
    }
    # vnat holds both heads: [128 k, hl, b, kt, HEAD+1]; column HEAD is the
    # ones column whose PV row yields the softmax denominator.
    vnat = persist.tile(
        [128, HPC, B, NKT, HEAD + 1], BF16, tag="vnat", name="vnat"
    )
    nc.vector.memset(vnat[:, :, :, :, HEAD:HEAD + 1], 1.0)
    ostage = persist.tile([128, B, NKT, OC], F32, tag="ostage", name="ostage")

    # ---------------- projection + LN machinery ----------------
    state = {}
    aptr = [0]  # position in AORDER for prefetch

    def emit_proj(n, c):
        # prefetch the xt chunk PREFETCH A-steps ahead
        i = aptr[0]
        aptr[0] += 1
        assert AORDER[i] == (n, c), (i, AORDER[i], (n, c))
        if i + PREFETCH < len(AORDER):
            load_xt(*AORDER[i + PREFETCH])
        xt = xt_tiles[(n, c)]
        ps_y = ps_y_pool.tile([128, TCH], F32, tag="ps_y", name="ps_y")
        for cc in range(NCC):
            nc.tensor.matmul(
                ps_y, lhsT=wt_sb[n][:, cc, :],
                rhs=xt[:, cc, :],
                start=(cc == 0), stop=(cc == NCC - 1),
            )
        # yt stays alive from the proj fill step until its (lagged) norm
        # fill step — a too-small ring creates a PE->DVE->ACT->PE circular
        # wait (deadlock), so keep generous slack over the ~4 alive.
        yt = p1tmp.tile([128, TCH], BF16, tag="yt", name="yt", bufs=8)
        if flags[f"bias_{n}"]:
            nc.vector.tensor_scalar(
                out=yt, in0=ps_y, scalar1=extra[f"bcol_{n}"],
                scalar2=None, op0=ALU.add,
            )
        else:
            nc.vector.tensor_copy(out=yt, in_=ps_y)
        state[(n, c)] = yt

    def emit_norm(n, c):
        # stats + norm in one step: sq, var matmul, rstd via ln/exp (same ACT
        # table as the attention exp stream), then the normalize multiply.
        yt = state.pop((n, c))
        sq = p1tmp.tile([128, TCH], BF16, tag="sq", name="sq")
        nc.vector.tensor_mul(sq, yt, yt)
        # var shares the ps_y ring (A and B steps alternate; keeps the misc
        # bank free for the vtrans/fin_b transpose pairs).
        ps_var = ps_y_pool.tile([128, TCH], F32, tag="ps_y", name="ps_var")
        nc.tensor.matmul(ps_var, lhsT=bones16, rhs=sq, start=True, stop=True)
        # rstd = exp(-0.5*ln(var + eps)); q also folds the 1/sqrt(HEAD) score
        # scale: ln(HEAD*var + HEAD*eps) adds ln(HEAD), so exp(-0.5*...)
        # yields rstd/sqrt(HEAD).
        lnv = p1tmp.tile([128, TCH], F32, tag="lnv", name="lnv")
        if n == "q":
            nc.scalar.activation(lnv, ps_var, AF.Ln, bias=eps_q, scale=float(HEAD))
        else:
            nc.scalar.activation(lnv, ps_var, AF.Ln, bias=eps_kv, scale=1.0)
        s_t = p1tmp.tile([128, TCH], F32, tag="s_t", name="s_t")
        nc.scalar.activation(s_t, lnv, AF.Exp, scale=-0.5)
        tsl = slice(c * TCH, (c + 1) * TCH)
        if flags[f"gb_{n}"]:
            lnf = p1tmp.tile([128, TCH], F32, tag="lnf", name="lnf")
            nc.vector.tensor_mul(lnf, yt, s_t)
            nc.vector.tensor_scalar(
                out=ln_sb[n][:, tsl], in0=lnf,
                scalar1=extra[f"gcol_{n}"], scalar2=extra[f"betacol_{n}"],
                op0=ALU.mult, op1=ALU.add,
            )
        else:
            nc.vector.tensor_mul(ln_sb[n][:, tsl], yt, s_t)

    def emit_vtrans(b, kt, hl):
        # one V tile back to natural layout: ln_v[64d, 128k] -> vnat[128k, 64d]
        # (a matmul/transpose with start=True zeroes the whole 2KB PSUM bank
        # region, so each transpose gets its own tile at offset 0).
        t0 = b * S
        dsl = slice(HEAD * hl, HEAD * (hl + 1))
        ps_tr = ps_misc_pool.tile([128, HEAD], BF16, tag="misc", name="ps_tr")
        nc.tensor.transpose(
            ps_tr,
            ln_sb["v"][dsl, t0 + kt * 128: t0 + (kt + 1) * 128],
            id16[dsl, dsl],
        )
        nc.vector.tensor_copy(out=vnat[:, hl, b, kt, 0:HEAD], in_=ps_tr)

    # ---------------- attention machinery ----------------
    pend = {"pv": None, "fin": None}

    def emit_pv(pu, kt):
        for hl in range(HPC):
            nc.tensor.matmul(
                pu["ps_o"][hl],
                lhsT=vnat[:, hl, pu["b"], kt, :],
                rhs=pu["at_q"][:, kt, hl, :],
                start=(kt == 0), stop=(kt == NKT - 1),
            )

    def emit_fin_a_hl(pu, hl):
        # DVE-only stage: drain ps_o[hl] into oT bf16 (rows 0..63 = out^T
        # values, row 64 = 1/den; row 65 pads the transpose to an even
        # bf16 element count and is never read). Frees ps_o quickly.
        oT = p3tmp.tile([HEAD + 2, QCH], BF16, tag="oT", name="oT", bufs=4)
        nc.vector.tensor_copy(out=oT[:HEAD, :], in_=pu["ps_o"][hl][:HEAD, :])
        # reciprocal_approx_fast requires base_partition 0 on both
        # operands (HW uop quirk) — stage the den row through base 0.
        den = p3tmp.tile([1, QCH], F32, tag="den", name="den", bufs=2)
        nc.vector.tensor_copy(out=den, in_=pu["ps_o"][hl][HEAD:HEAD + 1, :])
        inv = p3tmp.tile([1, QCH], F32, tag="inv", name="inv", bufs=2)
        nc.vector.reciprocal_approx_fast(inv, den)
        nc.vector.tensor_copy(out=oT[HEAD:HEAD + 1, :], in_=inv)
        pu.setdefault("oT", {})[hl] = oT

    def emit_fin_a(pu):
        for hl in range(HPC):
            emit_fin_a_hl(pu, hl)

    def emit_fin_b_item(pu, item, pool=None):
        # one transpose + normalize: item indexes (hl, sub)
        hl, sub = item // (QCH // 128), item % (QCH // 128)
        b, qc = pu["b"], pu["qc"]
        if pool is None:
            pool = ps_misc_pool
            tag = "misc"
        else:
            tag = "wave"
        ps_tro = pool.tile([128, HEAD + 2], BF16, tag=tag, name="ps_tro")
        nc.tensor.transpose(
            ps_tro, pu["oT"][hl][:, sub * 128:(sub + 1) * 128],
            id16[:HEAD + 2, :HEAD + 2],
        )
        # tensor_scalar needs an f32 scalar; stage the bf16 inv column
        inv_col = p3tmp.tile([128, 1], F32, tag="invc", name="inv_col", bufs=2)
        nc.vector.tensor_copy(out=inv_col, in_=ps_tro[:, HEAD:HEAD + 1])
        nc.vector.tensor_scalar(
            out=ostage[:, b, qc * (QCH // 128) + sub, HEAD * hl:HEAD * (hl + 1)],
            in0=ps_tro[:, 0:HEAD],
            scalar1=inv_col,
            scalar2=None, op0=ALU.mult,
        )

    N_FIN = HPC * (QCH // 128)  # 8 transpose+normalize items per unit

    def emit_store(pu):
        b, qc = pu["b"], pu["qc"]
        nsl = slice(qc * (QCH // 128), (qc + 1) * (QCH // 128))
        dst = aps["out"][:, b, :].rearrange("(n p) c -> p n c", p=128)[:, nsl, :]
        nc.sync.dma_start(out=dst, in_=ostage[:, b, nsl, :])

    def attn_unit(b, qc, vt_items=(), fills=()):
        t0 = b * S
        at_q = attn_pool.tile(
            [128, NKT, HPC, QCH], BF16, tag="at", name="at_q"
        )
        if pend["pv"] is not None:
            pend["pv"]["ps_o"] = [
                ps_o_pool.tile(
                    [HEAD + 1, QCH], F32, tag="ps_o", name="ps_o"
                )
                for _ in range(HPC)
            ]
        vt_items = list(vt_items)   # (min_wave, b, kt) pairs
        fills = list(fills)
        for kt in range(NKT):
            ps_wave = ps_wave_pool.tile(
                [128, HPC, QCH], F32, tag="wave", name="ps_wave"
            )
            for hl in range(HPC):
                dsl = slice(HEAD * hl, HEAD * (hl + 1))
                nc.tensor.matmul(
                    ps_wave[:, hl, :],
                    lhsT=ln_sb["k"][dsl, t0 + kt * 128: t0 + (kt + 1) * 128],
                    rhs=ln_sb["q"][dsl, t0 + qc * QCH: t0 + (qc + 1) * QCH],
                    start=True, stop=True,
                )
            nc.scalar.activation(at_q[:, kt], ps_wave, AF.Exp)
            if pend["pv"] is not None:
                emit_pv(pend["pv"], kt)
            if pend["fin"] is not None and kt < N_FIN:
                emit_fin_b_item(pend["fin"], kt)
            done = 0
            while done < 4 and vt_items and vt_items[0][0] <= kt:
                _, vb, vkt, vhl = vt_items.pop(0)
                emit_vtrans(vb, vkt, vhl)
                done += 1
            if fills:
                fills.pop(0)()
        assert not vt_items, vt_items
        assert not fills
        if pend["fin"] is not None:
            emit_store(pend["fin"])
        if pend["pv"] is not None:
            emit_fin_a(pend["pv"])
        pend["fin"] = pend["pv"]
        pend["pv"] = {"b": b, "qc": qc, "at_q": at_q}

    # ---------------- fused schedule ----------------
    def A(n, c):
        return lambda: emit_proj(n, c)

    def Bs(n, c):
        return lambda: emit_norm(n, c)

    # phase A: the minimum needed for attention unit (b0, qc0) to start.
    emit_proj("k", 0)
    emit_proj("k", 1)
    emit_norm("k", 0)
    emit_proj("q", 0)
    emit_norm("k", 1)
    emit_norm("q", 0)

    # per-unit fill lists (1 consumed per wave) and vtrans pair windows
    def vt(min_wave, b, kts):
        # per-(kt, hl) vtrans items, 4 consumed per wave from min_wave on
        return [
            (min_wave + i // 2, b, kt, hl)
            for i, kt in enumerate(kts) for hl in range(HPC)
        ]

    UNITS = [
        # (b, qc, fills, vtrans items as (min_wave, b, kt, hl))
        (0, 0,
         [A("q", 1), A("k", 2), A("v", 0), Bs("q", 1), A("k", 3),
          Bs("k", 2), A("v", 1), Bs("v", 0), Bs("k", 3), Bs("v", 1)],
         vt(9, 0, [0, 1, 2, 3]) + vt(11, 0, [4, 5, 6, 7])),
        (0, 1,
         [A("v", 2), A("v", 3), Bs("v", 2), Bs("v", 3), A("q", 2),
          Bs("q", 2)],
         vt(4, 0, [8, 9, 10, 11]) + vt(6, 0, [12, 13, 14, 15])),
        (0, 2,
         [A("q", 3), A("k", 4), Bs("q", 3), A("k", 5), Bs("k", 4),
          Bs("k", 5)],
         []),
        (0, 3,
         [A("q", 4), A("k", 6), Bs("q", 4), A("k", 7), Bs("k", 6),
          A("v", 4), Bs("k", 7), A("v", 5), Bs("v", 4), Bs("v", 5)],
         []),
        (1, 0,
         [A("q", 5), A("v", 6), Bs("q", 5), A("v", 7), Bs("v", 6),
          Bs("v", 7)],
         vt(0, 1, [0, 1, 2, 3]) + vt(2, 1, [4, 5, 6, 7])),
        (1, 1,
         [A("q", 6), Bs("q", 6)],
         vt(0, 1, [8, 9, 10, 11]) + vt(2, 1, [12, 13, 14, 15])),
        (1, 2, [A("q", 7), Bs("q", 7)], []),
        (1, 3, [], []),
    ]
    for b, qc, fills, vts in UNITS:
        attn_unit(b, qc, vt_items=vts, fills=fills)

    # drain: PV + finish of the last two units. The last unit's transposes
    # use the (now idle) ps_wave slots so consecutive pairs double-buffer,
    # and each head's transposes start as soon as that head's oT is drained.
    pend["pv"]["ps_o"] = [
        ps_o_pool.tile([HEAD + 1, QCH], F32, tag="ps_o", name="ps_o")
        for _ in range(HPC)
    ]
    for kt in range(NKT):
        emit_pv(pend["pv"], kt)
        if pend["fin"] is not None and kt < N_FIN:
            emit_fin_b_item(pend["fin"], kt)
    if pend["fin"] is not None:
        emit_store(pend["fin"])
    emit_fin_a_hl(pend["pv"], 0)
    for item in range(N_FIN // 2):
        if item == 0:
            emit_fin_a_hl(pend["pv"], 1)
        emit_fin_b_item(pend["pv"], item, pool=ps_wave_pool)
    for item in range(N_FIN // 2, N_FIN):
        emit_fin_b_item(pend["pv"], item, pool=ps_wave_pool)
    emit_store(pend["pv"])

    stack.close()


def _build(flags_key, flags, input_specs):
    nc = bacc.Bacc("TRN2", target_bir_lowering=False, debug=False)
    aps = {}
    for name, shape, dt in input_specs:
        aps[name] = nc.dram_tensor(name, list(shape), dt, kind="ExternalInput").ap()
    aps["out"] = nc.dram_tensor("out", [S, B, OC], F32, kind="ExternalOutput").ap()
    with tile.TileContext(nc) as tc:
        _emit(tc, aps, flags)
    nc.compile()
    return nc


_CACHE = {}


def kernel(**inputs):
    global LAST_RESULT
    bf16 = ml_dtypes.bfloat16
    f32 = np.float32

    Q, K, V = (np.asarray(inputs[n], f32) for n in ("Q", "K", "V"))
    W = {n: np.asarray(inputs["W" + n.upper()], f32) for n in ("q", "k", "v")}
    bias = {n: np.asarray(inputs["b" + n.upper()], f32) for n in ("q", "k", "v")}
    g = {n: np.asarray(inputs["g" + n.upper()], f32) for n in ("q", "k", "v")}
    beta = {n: np.asarray(inputs["beta" + n.upper()], f32) for n in ("q", "k", "v")}

    # X^T [c, t] with t = b*S + s
    xt = {
        "q": np.ascontiguousarray(Q.transpose(2, 1, 0).reshape(DIM, T)).astype(bf16),
        "k": np.ascontiguousarray(K.transpose(2, 1, 0).reshape(DIM, T)).astype(bf16),
        "v": np.ascontiguousarray(V.transpose(2, 1, 0).reshape(DIM, T)).astype(bf16),
    }
    blockones = np.kron(np.eye(2, dtype=f32), np.ones((HEAD, HEAD), f32)) / HEAD
    ident = np.eye(128, dtype=f32)

    flags = {}
    for n in ("q", "k", "v"):
        flags[f"bias_{n}"] = bool(np.any(bias[n] != 0.0))
        flags[f"gb_{n}"] = bool(np.any(g[n] != 1.0) or np.any(beta[n] != 0.0))
    flags_key = tuple(sorted(flags.items()))

    # per-core input maps
    in_maps = []
    shared = {
        "xt_q": xt["q"], "xt_k": xt["k"], "xt_v": xt["v"],
        "blockones_bf16": blockones.astype(bf16),
        "identity_bf16": ident.astype(bf16),
    }
    for p in range(NCORES):
        sl = slice(OC * p, OC * (p + 1))
        m = dict(shared)
        for n in ("q", "k", "v"):
            # fold the per-head mean subtraction into the weights: the 64
            # output rows of each head get their column-mean subtracted, so
            # the projection output is already centered.
            Wc = W[n][sl].astype(np.float64)
            Wc = Wc - Wc.reshape(HPC, HEAD, DIM).mean(axis=1, keepdims=True).repeat(
                HEAD, axis=1
            ).reshape(OC, DIM)
            # stationary W^T prepacked as [128, NCC, OC] (partition = c within
            # chunk) so the weight load is one contiguous 2KB-per-line DMA.
            wt = np.ascontiguousarray(
                Wc.T.reshape(NCC, 128, OC).transpose(1, 0, 2)
            ).astype(bf16)
            m[f"wt_{n}"] = wt
            if flags[f"bias_{n}"]:
                bc = bias[n][sl].astype(np.float64)
                bc = bc - bc.reshape(HPC, HEAD).mean(axis=1, keepdims=True).repeat(
                    HEAD, axis=1
                ).reshape(OC)
                m[f"bcol_{n}"] = bc.astype(f32).reshape(128, 1)
            if flags[f"gb_{n}"]:
                m[f"gcol_{n}"] = np.tile(g[n], HPC).astype(f32).reshape(128, 1)
                bcol = np.tile(beta[n], HPC).astype(f32)
                if n == "q":
                    bcol = (bcol / np.sqrt(HEAD)).astype(f32)
                m[f"betacol_{n}"] = bcol.reshape(128, 1)
        in_maps.append(m)

    if flags_key not in _CACHE:
        input_specs = []
        for name, arr in in_maps[0].items():
            dt = BF16 if arr.dtype == bf16 else F32
            input_specs.append((name, arr.shape, dt))
        _CACHE[flags_key] = _build(flags_key, flags, input_specs)
    nc = _CACHE[flags_key]

    trace = bool(os.environ.get("KERNEL_TRACE"))
    tmpdir = os.environ.get("KERNEL_TRACE_DIR") or None
    res = run_bass_kernel_spmd(
        nc, in_maps, core_ids=list(range(NCORES)), trace=trace, tmpdir=tmpdir
    )
    LAST_RESULT = res
    out = np.concatenate(
        [np.asarray(res.results[p]["out"], f32) for p in range(NCORES)], axis=2
    )
    return out


# revision 13
# speedup vs baseline: 1.0887x; 1.0015x over previous
"""Trainium2 Bass kernel for nn_CrossAttention (per-head-LN cross attention).

Sharding: 16 heads / 8 cores -> 2 heads per core, both batch elements on every
core (attention is embarrassingly parallel over (B, H)). Each core computes its
128 output channels [128p, 128p+128) of the final [S, B, 1024] output.

Device algorithm (per core, all matmuls bf16 with f32 PSUM accumulation):
  - Projections computed transposed: Y^T[o, t] (o = core's 128 channels,
    t = b*S + s), via stationary W^T chunks against streamed X^T tiles.
  - Per-head mean subtraction is folded into the weights on the host
    (W' = W - per-head column mean), so Y^T is already centered: no mu
    matmul or subtract on device.  Variance via matmul-broadcast:
    var_bc = blockones.T @ yt^2 (bf16), rstd = exp(-0.5 ln(var+eps)) on ACT.
    The 1/sqrt(head) score scale is folded into Q's rstd.
  - V transposed back to natural [k, d] per (b,h) via PE transposes, with a
    ones column appended (row 64 of the PV output then holds the softmax
    denominator).
  - scores^T[k, q] = K^T.T @ Q^T per (b,h); softmax without max subtraction
    (scores are O(1) after LN; exp cannot overflow); exp on ACT directly from
    PSUM; PV: out^T[d|den, q] accumulated over k chunks; PE-transpose (bf16)
    back to natural [q, d] and multiply by 1/den.
  - Schedule: attention unit (b0,qc0) starts as soon as K chunk0/1 + Q chunk0
    are normed (~8us in).  ALL remaining 21 projection chunks ride as fills
    inside the 8 attention units (1 fill step per wave), with per-chunk DMA
    prefetch 4 A-steps ahead.  V transposes are emitted as per-kt pairs
    (2 PE transposes + 1 DVE copy into a [128, 2hl, ...] vnat tile) gated on
    an earliest-wave so they always follow their chunk's norm in DVE order.
    Output finish transposes are batched in pairs per wave.
"""

import os
import numpy as np
import ml_dtypes

import concourse.bacc as bacc
import concourse.mybir as mybir
import concourse.tile as tile
from concourse.bass_utils import run_bass_kernel_spmd

F32 = mybir.dt.float32
BF16 = mybir.dt.bfloat16
AF = mybir.ActivationFunctionType
ALU = mybir.AluOpType

S = 2048
B = 2
DIM = 1024
NHEAD = 16
HEAD = 64
EPS = 1e-5
NCORES = 8
OC = DIM // NCORES          # 128 output channels per core
HPC = OC // HEAD            # 2 heads per core
T = S * B                   # 4096 tokens (t = b*S + s)
TCH = 512                   # token chunk (matmul moving free dim)
NT = T // TCH               # 8 token chunks per tensor (0-3 = b0, 4-7 = b1)
NCC = DIM // 128            # 8 contraction chunks
QCH = 512
NQ = S // QCH               # 4 q chunks per (b, h)
NKT = S // 128              # 16 k tiles per (b, h)

LAST_RESULT = None

# global A-step order (for DMA prefetch): phase A then per-unit fills
AORDER = [
    ("k", 0), ("q", 0), ("k", 1),                         # phase A
    ("q", 1), ("k", 2), ("v", 0), ("k", 3), ("v", 1),      # u0
    ("v", 2), ("v", 3), ("q", 2),                          # u1
    ("q", 3), ("k", 4), ("k", 5),                          # u2
    ("q", 4), ("k", 6), ("k", 7), ("v", 4), ("v", 5),      # u3
    ("q", 5), ("v", 6), ("v", 7),                          # u4
    ("q", 6),                                              # u5
    ("q", 7),                                              # u6
]
PREFETCH = 4


def _emit(tc, aps, flags):
    from contextlib import ExitStack

    nc = tc.nc
    names = ("q", "k", "v")

    stack = ExitStack()
    consts = stack.enter_context(tc.tile_pool(name="consts", bufs=1))
    persist = stack.enter_context(tc.tile_pool(name="persist", bufs=1))
    xload = stack.enter_context(tc.tile_pool(name="xload", bufs=5))
    p1tmp = stack.enter_context(tc.tile_pool(name="p1tmp", bufs=2))
    lnp = stack.enter_context(tc.tile_pool(name="lnpool", bufs=1))
    attn_pool = stack.enter_context(tc.tile_pool(name="attn", bufs=2))
    p3tmp = stack.enter_context(tc.tile_pool(name="p3tmp", bufs=1))
    ps_y_pool = stack.enter_context(tc.tile_pool(name="ps_y", bufs=1, space="PSUM"))
    ps_wave_pool = stack.enter_context(
        tc.tile_pool(name="ps_wave", bufs=2, space="PSUM")
    )
    ps_o_pool = stack.enter_context(tc.tile_pool(name="ps_o", bufs=2, space="PSUM"))
    ps_misc_pool = stack.enter_context(
        tc.tile_pool(name="ps_misc", bufs=1, space="PSUM")
    )

    # Pin the ACT function table to the set containing BOTH Exp and Ln so
    # the auto-inserter never ping-pongs 1.3us table reloads between the
    # attention exp stream and the LN rstd ln/exp ops.
    from concourse.hw_specs import get_activation_tables

    tabs = get_activation_tables(nc.m.arch)
    combined_idx = None
    for i, (_name, fns) in enumerate(tabs.items()):
        if AF.Exp in fns and AF.Ln in fns:
            combined_idx = i
            break
    assert combined_idx is not None, "no ACT table with both Exp and Ln"
    nc.scalar.add_instruction(
        mybir.InstLoadActFuncSet(
            name=nc.get_next_instruction_name(),
            act_func_set_id=combined_idx,
            ins=[], outs=[],
        )
    )

    # ---------------- constants + critical-path input loads ----------------
    xt_tiles = {}

    def load_xt(n, c):
        if (n, c) in xt_tiles:
            return
        xt = xload.tile([128, NCC, TCH], BF16, tag="xt", name="xt")
        tsl = slice(c * TCH, (c + 1) * TCH)
        nc.sync.dma_start(
            out=xt,
            in_=aps[f"xt_{n}"][:, tsl].rearrange("(a p) t -> p a t", p=128),
        )
        xt_tiles[(n, c)] = xt

    wt_sb = {}

    def load_wt(n):
        t = consts.tile([128, NCC, OC], BF16, tag=f"wt_{n}", name=f"wt_{n}")
        nc.sync.dma_start(out=t, in_=aps[f"wt_{n}"])
        wt_sb[n] = t

    # DMA issue order tracks first use: k chunk0 + k weights gate the first
    # projection matmul, then q0 for the first attention wave.
    load_wt("k")
    load_xt("k", 0)
    bones16 = consts.tile([128, OC], BF16, tag="bones16", name="bones16")
    nc.sync.dma_start(out=bones16, in_=aps["blockones_bf16"])
    load_wt("q")
    load_xt("q", 0)
    id16 = consts.tile([128, 128], BF16, tag="id16", name="id16")
    nc.sync.dma_start(out=id16, in_=aps["identity_bf16"])
    load_xt("k", 1)
    load_wt("v")
    load_xt("q", 1)
    eps_q = consts.tile([128, 1], F32, tag="eps_q", name="eps_q")
    nc.vector.memset(eps_q, float(HEAD * EPS))
    eps_kv = consts.tile([128, 1], F32, tag="eps_kv", name="eps_kv")
    nc.vector.memset(eps_kv, float(EPS))
    extra = {}
    for n in names:
        if flags[f"bias_{n}"]:
            t = consts.tile([128, 1], F32, tag=f"bcol_{n}", name=f"bcol_{n}")
            nc.sync.dma_start(out=t, in_=aps[f"bcol_{n}"])
            extra[f"bcol_{n}"] = t
        if flags[f"gb_{n}"]:
            tg = consts.tile([128, 1], F32, tag=f"gcol_{n}", name=f"gcol_{n}")
            nc.sync.dma_start(out=tg, in_=aps[f"gcol_{n}"])
            tb = consts.tile([128, 1], F32, tag=f"betacol_{n}", name=f"betacol_{n}")
            nc.sync.dma_start(out=tb, in_=aps[f"betacol_{n}"])
            extra[f"gcol_{n}"] = tg
            extra[f"betacol_{n}"] = tb

    # ---------------- persistent tiles ----------------
    ln_sb = {
        n: lnp.tile([128, T], BF16, tag=f"ln_{n}", name=f"ln_{n}") for n in names
    }
    # vnat holds both heads: [128 k, hl, b, kt, HEAD+1]; column HEAD is the
    # ones column whose PV row yields the softmax denominator.
    vnat = persist.tile(
        [128, HPC, B, NKT, HEAD + 1], BF16, tag="vnat", name="vnat"
    )
    nc.vector.memset(vnat[:, :, :, :, HEAD:HEAD + 1], 1.0)
    ostage = persist.tile([128, B, NKT, OC], F32, tag="ostage", name="ostage")

    # ---------------- projection + LN machinery ----------------
    state = {}
    aptr = [0]  # position in AORDER for prefetch

    def emit_proj(n, c):
        # prefetch the xt chunk PREFETCH A-steps ahead
        i = aptr[0]
        aptr[0] += 1
        assert AORDER[i] == (n, c), (i, AORDER[i], (n, c))
        if i + PREFETCH < len(AORDER):
            load_xt(*AORDER[i + PREFETCH])
        xt = xt_tiles[(n, c)]
        ps_y = ps_y_pool.tile([128, TCH], F32, tag="ps_y", name="ps_y")
        for cc in range(NCC):
            nc.tensor.matmul(
                ps_y, lhsT=wt_sb[n][:, cc, :],
                rhs=xt[:, cc, :],
                start=(cc == 0), stop=(cc == NCC - 1),
            )
        # yt stays alive from the proj fill step until its (lagged) norm
        # fill step — a too-small ring creates a PE->DVE->ACT->PE circular
        # wait (deadlock), so keep generous slack over the ~4 alive.
        yt = p1tmp.tile([128, TCH], BF16, tag="yt", name="yt", bufs=8)
        if flags[f"bias_{n}"]:
            nc.vector.tensor_scalar(
                out=yt, in0=ps_y, scalar1=extra[f"bcol_{n}"],
                scalar2=None, op0=ALU.add,
            )
        else:
            nc.vector.tensor_copy(out=yt, in_=ps_y)
        state[(n, c)] = yt

    def emit_norm(n, c):
        # stats + norm in one step: sq, var matmul, rstd via ln/exp (same ACT
        # table as the attention exp stream), then the normalize multiply.
        yt = state.pop((n, c))
        sq = p1tmp.tile([128, TCH], BF16, tag="sq", name="sq")
        nc.vector.tensor_mul(sq, yt, yt)
        # var shares the ps_y ring (A and B steps alternate); DVE drains it to
        # SBUF immediately so the ring is never gated on the busy ACT engine.
        ps_var = ps_y_pool.tile([128, TCH], F32, tag="ps_y", name="ps_var")
        nc.tensor.matmul(ps_var, lhsT=bones16, rhs=sq, start=True, stop=True)
        var_sb = p1tmp.tile([128, TCH], F32, tag="var_sb", name="var_sb")
        nc.vector.tensor_copy(out=var_sb, in_=ps_var)
        # rstd = exp(-0.5*ln(var + eps)); q also folds the 1/sqrt(HEAD) score
        # scale: ln(HEAD*var + HEAD*eps) adds ln(HEAD), so exp(-0.5*...)
        # yields rstd/sqrt(HEAD).
        lnv = p1tmp.tile([128, TCH], F32, tag="lnv", name="lnv")
        if n == "q":
            nc.scalar.activation(lnv, var_sb, AF.Ln, bias=eps_q, scale=float(HEAD))
        else:
            nc.scalar.activation(lnv, var_sb, AF.Ln, bias=eps_kv, scale=1.0)
        s_t = p1tmp.tile([128, TCH], F32, tag="s_t", name="s_t")
        nc.scalar.activation(s_t, lnv, AF.Exp, scale=-0.5)
        tsl = slice(c * TCH, (c + 1) * TCH)
        if flags[f"gb_{n}"]:
            lnf = p1tmp.tile([128, TCH], F32, tag="lnf", name="lnf")
            nc.vector.tensor_mul(lnf, yt, s_t)
            nc.vector.tensor_scalar(
                out=ln_sb[n][:, tsl], in0=lnf,
                scalar1=extra[f"gcol_{n}"], scalar2=extra[f"betacol_{n}"],
                op0=ALU.mult, op1=ALU.add,
            )
        else:
            nc.vector.tensor_mul(ln_sb[n][:, tsl], yt, s_t)

    def emit_vtrans(b, kt, hl):
        # one V tile back to natural layout: ln_v[64d, 128k] -> vnat[128k, 64d]
        # (a matmul/transpose with start=True zeroes the whole 2KB PSUM bank
        # region, so each transpose gets its own tile at offset 0).
        t0 = b * S
        dsl = slice(HEAD * hl, HEAD * (hl + 1))
        ps_tr = ps_misc_pool.tile([128, HEAD], BF16, tag="misc", name="ps_tr")
        nc.tensor.transpose(
            ps_tr,
            ln_sb["v"][dsl, t0 + kt * 128: t0 + (kt + 1) * 128],
            id16[dsl, dsl],
        )
        nc.vector.tensor_copy(out=vnat[:, hl, b, kt, 0:HEAD], in_=ps_tr)

    # ---------------- attention machinery ----------------
    pend = {"pv": None, "fin": None}

    def emit_pv(pu, kt):
        for hl in range(HPC):
            nc.tensor.matmul(
                pu["ps_o"][hl],
                lhsT=vnat[:, hl, pu["b"], kt, :],
                rhs=pu["at_q"][:, kt, hl, :],
                start=(kt == 0), stop=(kt == NKT - 1),
            )

    def emit_fin_a_hl(pu, hl):
        # DVE-only stage: drain ps_o[hl] into oT bf16 (rows 0..63 = out^T
        # values, row 64 = 1/den; row 65 pads the transpose to an even
        # bf16 element count and is never read). Frees ps_o quickly.
        oT = p3tmp.tile([HEAD + 2, QCH], BF16, tag="oT", name="oT", bufs=4)
        nc.vector.tensor_copy(out=oT[:HEAD, :], in_=pu["ps_o"][hl][:HEAD, :])
        # reciprocal_approx_fast requires base_partition 0 on both
        # operands (HW uop quirk) — stage the den row through base 0.
        den = p3tmp.tile([1, QCH], F32, tag="den", name="den", bufs=2)
        nc.vector.tensor_copy(out=den, in_=pu["ps_o"][hl][HEAD:HEAD + 1, :])
        inv = p3tmp.tile([1, QCH], F32, tag="inv", name="inv", bufs=2)
        nc.vector.reciprocal_approx_fast(inv, den)
        nc.vector.tensor_copy(out=oT[HEAD:HEAD + 1, :], in_=inv)
        pu.setdefault("oT", {})[hl] = oT

    def emit_fin_a(pu):
        for hl in range(HPC):
            emit_fin_a_hl(pu, hl)

    def emit_fin_b_item(pu, item, pool=None):
        # one transpose + normalize: item indexes (hl, sub)
        hl, sub = item // (QCH // 128), item % (QCH // 128)
        b, qc = pu["b"], pu["qc"]
        if pool is None:
            pool = ps_misc_pool
            tag = "misc"
        else:
            tag = "wave"
        ps_tro = pool.tile([128, HEAD + 2], BF16, tag=tag, name="ps_tro")
        nc.tensor.transpose(
            ps_tro, pu["oT"][hl][:, sub * 128:(sub + 1) * 128],
            id16[:HEAD + 2, :HEAD + 2],
        )
        # tensor_scalar needs an f32 scalar; stage the bf16 inv column
        inv_col = p3tmp.tile([128, 1], F32, tag="invc", name="inv_col", bufs=2)
        nc.vector.tensor_copy(out=inv_col, in_=ps_tro[:, HEAD:HEAD + 1])
        nc.vector.tensor_scalar(
            out=ostage[:, b, qc * (QCH // 128) + sub, HEAD * hl:HEAD * (hl + 1)],
            in0=ps_tro[:, 0:HEAD],
            scalar1=inv_col,
            scalar2=None, op0=ALU.mult,
        )

    N_FIN = HPC * (QCH // 128)  # 8 transpose+normalize items per unit

    def emit_store(pu):
        b, qc = pu["b"], pu["qc"]
        nsl = slice(qc * (QCH // 128), (qc + 1) * (QCH // 128))
        dst = aps["out"][:, b, :].rearrange("(n p) c -> p n c", p=128)[:, nsl, :]
        nc.sync.dma_start(out=dst, in_=ostage[:, b, nsl, :])

    def attn_unit(b, qc, vt_items=(), fills=()):
        t0 = b * S
        at_q = attn_pool.tile(
            [128, NKT, HPC, QCH], BF16, tag="at", name="at_q"
        )
        if pend["pv"] is not None:
            pend["pv"]["ps_o"] = [
                ps_o_pool.tile(
                    [HEAD + 1, QCH], F32, tag="ps_o", name="ps_o"
                )
                for _ in range(HPC)
            ]
        vt_items = list(vt_items)   # (min_wave, b, kt) pairs
        fills = list(fills)
        for kt in range(NKT):
            # PV of the previous unit goes first: if the scores matmul stalls
            # on its psum slot (exp two waves back not drained), the PE still
            # has the independent PV work already issued ahead of it.
            if pend["pv"] is not None:
                emit_pv(pend["pv"], kt)
            ps_wave = ps_wave_pool.tile(
                [128, HPC, QCH], F32, tag="wave", name="ps_wave"
            )
            for hl in range(HPC):
                dsl = slice(HEAD * hl, HEAD * (hl + 1))
                nc.tensor.matmul(
                    ps_wave[:, hl, :],
                    lhsT=ln_sb["k"][dsl, t0 + kt * 128: t0 + (kt + 1) * 128],
                    rhs=ln_sb["q"][dsl, t0 + qc * QCH: t0 + (qc + 1) * QCH],
                    start=True, stop=True,
                )
            nc.scalar.activation(at_q[:, kt], ps_wave, AF.Exp)
            if pend["fin"] is not None and kt < N_FIN:
                emit_fin_b_item(pend["fin"], kt)
            done = 0
            while done < 4 and vt_items and vt_items[0][0] <= kt:
                _, vb, vkt, vhl = vt_items.pop(0)
                emit_vtrans(vb, vkt, vhl)
                done += 1
            if fills:
                fills.pop(0)()
        assert not vt_items, vt_items
        assert not fills
        if pend["fin"] is not None:
            emit_store(pend["fin"])
        if pend["pv"] is not None:
            emit_fin_a(pend["pv"])
        pend["fin"] = pend["pv"]
        pend["pv"] = {"b": b, "qc": qc, "at_q": at_q}

    # ---------------- fused schedule ----------------
    def A(n, c):
        return lambda: emit_proj(n, c)

    def Bs(n, c):
        return lambda: emit_norm(n, c)

    # phase A: the minimum needed for attention unit (b0, qc0) to start.
    # Norms directly follow their proj so the (monotonic per-engine) Tile
    # semaphores never make an early norm wait on a later chunk's matmuls.
    emit_proj("k", 0)
    emit_norm("k", 0)
    emit_proj("q", 0)
    emit_norm("q", 0)
    emit_proj("k", 1)
    emit_norm("k", 1)

    # per-unit fill lists (1 consumed per wave) and vtrans pair windows
    def vt(min_wave, b, kts):
        # per-(kt, hl) vtrans items, 4 consumed per wave from min_wave on
        return [
            (min_wave + i // 2, b, kt, hl)
            for i, kt in enumerate(kts) for hl in range(HPC)
        ]

    UNITS = [
        # (b, qc, fills, vtrans items as (min_wave, b, kt, hl))
        (0, 0,
         [A("q", 1), A("k", 2), A("v", 0), Bs("q", 1), A("k", 3),
          Bs("k", 2), A("v", 1), Bs("v", 0), Bs("k", 3), Bs("v", 1)],
         vt(9, 0, [0, 1, 2, 3]) + vt(11, 0, [4, 5, 6, 7])),
        (0, 1,
         [A("v", 2), A("v", 3), Bs("v", 2), Bs("v", 3), A("q", 2),
          Bs("q", 2)],
         vt(4, 0, [8, 9, 10, 11]) + vt(6, 0, [12, 13, 14, 15])),
        (0, 2,
         [A("q", 3), A("k", 4), Bs("q", 3), A("k", 5), Bs("k", 4),
          Bs("k", 5)],
         []),
        (0, 3,
         [A("q", 4), A("k", 6), Bs("q", 4), A("k", 7), Bs("k", 6),
          A("v", 4), Bs("k", 7), A("v", 5), Bs("v", 4), Bs("v", 5)],
         []),
        (1, 0,
         [A("q", 5), A("v", 6), Bs("q", 5), A("v", 7), Bs("v", 6),
          Bs("v", 7)],
         vt(0, 1, [0, 1, 2, 3]) + vt(2, 1, [4, 5, 6, 7])),
        (1, 1,
         [A("q", 6), Bs("q", 6)],
         vt(0, 1, [8, 9, 10, 11]) + vt(2, 1, [12, 13, 14, 15])),
        (1, 2, [A("q", 7), Bs("q", 7)], []),
        (1, 3, [], []),
    ]
    for b, qc, fills, vts in UNITS:
        attn_unit(b, qc, vt_items=vts, fills=fills)

    # drain: PV + finish of the last two units. The last unit's transposes
    # use the (now idle) ps_wave slots so consecutive pairs double-buffer,
    # and each head's transposes start as soon as that head's oT is drained.
    pend["pv"]["ps_o"] = [
        ps_o_pool.tile([HEAD + 1, QCH], F32, tag="ps_o", name="ps_o")
        for _ in range(HPC)
    ]
    for kt in range(NKT):
        emit_pv(pend["pv"], kt)
        if pend["fin"] is not None and kt < N_FIN:
            emit_fin_b_item(pend["fin"], kt)
    if pend["fin"] is not None:
        emit_store(pend["fin"])
    emit_fin_a_hl(pend["pv"], 0)
    for item in range(N_FIN // 2):
        if item == 0:
            emit_fin_a_hl(pend["pv"], 1)
        emit_fin_b_item(pend["pv"], item, pool=ps_wave_pool)
    for item in range(N_FIN // 2, N_FIN):
        emit_fin_b_item(pend["pv"], item, pool=ps_wave_pool)
    emit_store(pend["pv"])

    stack.close()


def _build(flags_key, flags, input_specs):
    nc = bacc.Bacc("TRN2", target_bir_lowering=False, debug=False)
    aps = {}
    for name, shape, dt in input_specs:
        aps[name] = nc.dram_tensor(name, list(shape), dt, kind="ExternalInput").ap()
    aps["out"] = nc.dram_tensor("out", [S, B, OC], F32, kind="ExternalOutput").ap()
    with tile.TileContext(nc) as tc:
        _emit(tc, aps, flags)
    nc.compile()
    return nc


_CACHE = {}


def kernel(**inputs):
    global LAST_RESULT
    bf16 = ml_dtypes.bfloat16
    f32 = np.float32

    Q, K, V = (np.asarray(inputs[n], f32) for n in ("Q", "K", "V"))
    W = {n: np.asarray(inputs["W" + n.upper()], f32) for n in ("q", "k", "v")}
    bias = {n: np.asarray(inputs["b" + n.upper()], f32) for n in ("q", "k", "v")}
    g = {n: np.asarray(inputs["g" + n.upper()], f32) for n in ("q", "k", "v")}
    beta = {n: np.asarray(inputs["beta" + n.upper()], f32) for n in ("q", "k", "v")}

    # X^T [c, t] with t = b*S + s
    xt = {
        "q": np.ascontiguousarray(Q.transpose(2, 1, 0).reshape(DIM, T)).astype(bf16),
        "k": np.ascontiguousarray(K.transpose(2, 1, 0).reshape(DIM, T)).astype(bf16),
        "v": np.ascontiguousarray(V.transpose(2, 1, 0).reshape(DIM, T)).astype(bf16),
    }
    blockones = np.kron(np.eye(2, dtype=f32), np.ones((HEAD, HEAD), f32)) / HEAD
    ident = np.eye(128, dtype=f32)

    flags = {}
    for n in ("q", "k", "v"):
        flags[f"bias_{n}"] = bool(np.any(bias[n] != 0.0))
        flags[f"gb_{n}"] = bool(np.any(g[n] != 1.0) or np.any(beta[n] != 0.0))
    flags_key = tuple(sorted(flags.items()))

    # per-core input maps
    in_maps = []
    shared = {
        "xt_q": xt["q"], "xt_k": xt["k"], "xt_v": xt["v"],
        "blockones_bf16": blockones.astype(bf16),
        "identity_bf16": ident.astype(bf16),
    }
    for p in range(NCORES):
        sl = slice(OC * p, OC * (p + 1))
        m = dict(shared)
        for n in ("q", "k", "v"):
            # fold the per-head mean subtraction into the weights: the 64
            # output rows of each head get their column-mean subtracted, so
            # the projection output is already centered.
            Wc = W[n][sl].astype(np.float64)
            Wc = Wc - Wc.reshape(HPC, HEAD, DIM).mean(axis=1, keepdims=True).repeat(
                HEAD, axis=1
            ).reshape(OC, DIM)
            # stationary W^T prepacked as [128, NCC, OC] (partition = c within
            # chunk) so the weight load is one contiguous 2KB-per-line DMA.
            wt = np.ascontiguousarray(
                Wc.T.reshape(NCC, 128, OC).transpose(1, 0, 2)
            ).astype(bf16)
            m[f"wt_{n}"] = wt
            if flags[f"bias_{n}"]:
                bc = bias[n][sl].astype(np.float64)
                bc = bc - bc.reshape(HPC, HEAD).mean(axis=1, keepdims=True).repeat(
                    HEAD, axis=1
                ).reshape(OC)
                m[f"bcol_{n}"] = bc.astype(f32).reshape(128, 1)
            if flags[f"gb_{n}"]:
                m[f"gcol_{n}"] = np.tile(g[n], HPC).astype(f32).reshape(128, 1)
                bcol = np.tile(beta[n], HPC).astype(f32)
                if n == "q":
                    bcol = (bcol / np.sqrt(HEAD)).astype(f32)
                m[f"betacol_{n}"] = bcol.reshape(128, 1)
        in_maps.append(m)

    if flags_key not in _CACHE:
        input_specs = []
        for name, arr in in_maps[0].items():
            dt = BF16 if arr.dtype == bf16 else F32
            input_specs.append((name, arr.shape, dt))
        _CACHE[flags_key] = _build(flags_key, flags, input_specs)
    nc = _CACHE[flags_key]

    trace = bool(os.environ.get("KERNEL_TRACE"))
    tmpdir = os.environ.get("KERNEL_TRACE_DIR") or None
    res = run_bass_kernel_spmd(
        nc, in_maps, core_ids=list(range(NCORES)), trace=trace, tmpdir=tmpdir
    )
    LAST_RESULT = res
    out = np.concatenate(
        [np.asarray(res.results[p]["out"], f32) for p in range(NCORES)], axis=2
    )
    return out
